# revision 35
# baseline (speedup 1.0000x reference)
"""Trainium2 Bass kernel for nn_ActorNet (LSTM actor network).

Data-parallel across 8 NeuronCores: each core processes 128 batch rows.
Per core, the LSTM scan runs in a transposed layout (gate channels /
hidden dim on partitions, batch on the free dim) as two interleaved
64-row groups so the sequential per-step dependency chain of one group
hides under the engine work of the other.

Host-side prep (inside kernel(), before sharding):
  - x/previous_actions are transposed to [F, T*B_local] bf16 so device
    DMAs are contiguous >=4KB runs per partition row.
  - Wx/Wh/W1/W2/W3 are pre-cast to bf16; the LSTM bias b is appended as
    a 65th row of Wh (paired with an ones-row in h, K=65); the g-gate
    columns of Wx/Wh/b are pre-scaled by 2 for the sigmoid trick.
  - W1 is pre-rearranged to [h, (t, out)] for per-step accumulation.

Key device structure per core:
  - xproj (xs @ Wx) is computed chunk-by-chunk (4 timesteps) directly
    into the gate PSUM banks (bf16 matmuls, N=256).
  - The recurrent matmul h @ Wh accumulates on top (start=False).
  - All four gates go through ONE sigmoid ACT op per group-step
    ([128 partitions, 2 banks, 64 batch]); tanh(g) is recovered as
    2*sigmoid(2g)-1, with the correction fused into one custom-DVE op:
    i*g = (2*sg - 1) * si  (AFFINE_MUL_REDUCE body (in0*s0+s1)*in1).
  - The big head GEMM (flat hs @ W1) is accumulated per-step into a
    PSUM bank: a1T += W1[t].T @ h_t  (64x64x64 matmuls, bf16).
  - Heads: relu(a1+b1) -> W2 -> relu -> W3 -> +b3, in transposed layout
    with per-partition ACT biases; outputs transposed back via PE.
"""

import ml_dtypes
import numpy as np

import concourse.bass as bass
import concourse.bacc as bacc
import concourse.mybir as mybir
from concourse.tile import TileContext
from concourse.bass_utils import run_bass_kernel_spmd
from concourse.dve_ops import AFFINE_MUL_REDUCE
from concourse.masks import make_identity

B, T, F, A, H = 1024, 256, 124, 4, 64
FA = F + A            # 128 input features to the LSTM
G4 = 4 * H            # 256 gate channels
NCORES = 8
BL = B // NCORES      # 128 batch rows per core
NG = 2                # independent chain groups per core
BG = BL // NG         # 64 batch rows per group
CH = 4                # timesteps per xproj matmul chunk
DCH = 16              # timesteps per input DMA chunk
NCH = T // CH
OUTW = 2 * A + 2 * H  # packed output width: actor(8) | c(64) | h(64)

f32 = mybir.dt.float32
bf16 = mybir.dt.bfloat16
AF = mybir.ActivationFunctionType
BF = ml_dtypes.bfloat16


def build(T=T, DCH=DCH, NG=NG):
    BG = BL // NG
    NCH = T // CH
    nc = bacc.Bacc("TRN2", target_bir_lowering=False, debug=False, num_devices=NCORES)
    xpat_d = nc.declare_dram_parameter("xpat", [FA, T * BL], bf16, isOutput=False)
    Wx_d = nc.declare_dram_parameter("Wxp", [FA, G4], bf16, isOutput=False)
    Wh_d = nc.declare_dram_parameter("Whp", [H, G4], bf16, isOutput=False)
    b_d = nc.declare_dram_parameter("bp", [1, G4], bf16, isOutput=False)
    W1_d = nc.declare_dram_parameter("W1p", [H, T * H], bf16, isOutput=False)
    b1_d = nc.declare_dram_parameter("b1", [H], f32, isOutput=False)
    W2_d = nc.declare_dram_parameter("W2p", [H, H], bf16, isOutput=False)
    b2_d = nc.declare_dram_parameter("b2", [H], f32, isOutput=False)
    W3_d = nc.declare_dram_parameter("W3p", [H, 2 * A], bf16, isOutput=False)
    b3_d = nc.declare_dram_parameter("b3", [2 * A], f32, isOutput=False)
    out_d = nc.declare_dram_parameter("out", [BL, OUTW], f32, isOutput=True)

    with TileContext(nc) as tc:
        persist = tc.alloc_tile_pool(name="persist", bufs=1)

        # ---------------- weights / constants ----------------
        # The LSTM cell state and h live on partitions 64:128 ("hi"), the
        # same lanes as sigma(f) and sigma(o), so the multiplies that update
        # them are lane-aligned without any cross-partition moves on the
        # critical path. Weights feeding matmuls on h are loaded at base
        # partition 64 (PE row group 2-3 via tile_position).
        Wx_sb = persist.tile([FA, G4], bf16, name="Wx_sb")
        Whh = persist.tile([128, G4], bf16, name="Whh")
        b_sb = persist.tile([1, G4], bf16, name="b_sb")
        ones_sb = persist.tile([1, CH * BL], bf16, name="ones_sb")
        W1_sb = persist.tile([128, T * H], bf16, name="W1_sb")
        W2_sb = persist.tile([H, H], bf16, name="W2_sb")
        W3_sb = persist.tile([H, 2 * A], bf16, name="W3_sb")
        b1_sb = persist.tile([H, 1], f32, name="b1_sb")
        b2_sb = persist.tile([H, 1], f32, name="b2_sb")
        b3_sb = persist.tile([2 * A, 1], f32, name="b3_sb")
        ident = persist.tile([128, H], f32, name="ident")

        nc.sync.dma_start(out=Wx_sb[:], in_=Wx_d[:])
        nc.sync.dma_start(out=Whh[64:128, :], in_=Wh_d[:])
        nc.sync.dma_start(out=b_sb[:], in_=b_d[:])
        nc.sync.dma_start(out=W1_sb[64:128, :], in_=W1_d[:])
        nc.sync.dma_start(out=W2_sb[:], in_=W2_d[:])
        nc.sync.dma_start(out=W3_sb[:], in_=W3_d[:])
        nc.sync.dma_start(out=b1_sb[:], in_=b1_d[:, None])
        nc.sync.dma_start(out=b2_sb[:], in_=b2_d[:, None])
        nc.sync.dma_start(out=b3_sb[:], in_=b3_d[:, None])
        nc.vector.memset(ones_sb[:], 1.0)
        make_identity(nc, ident[0:H, :])
        make_identity(nc, ident[64:128, :])

        # ---------------- per-group state (hi partitions) ----------------
        h_ext = [[None, None] for _ in range(NG)]
        for g in range(NG):
            for j in range(2):
                h_ext[g][j] = persist.tile([128, BG], bf16, name=f"h_ext{g}_{j}")
                nc.vector.memset(h_ext[g][j][64:128, :], 0.0)
        c_all = persist.tile([128, BL], f32, name="c_all")
        nc.vector.memset(c_all[64:128, :], 0.0)
        hf_all = persist.tile([128, BL], f32, name="hf_all")
        c_t = [c_all[64:128, g * BG : (g + 1) * BG] for g in range(NG)]
        hf = [hf_all[64:128, g * BG : (g + 1) * BG] for g in range(NG)]

        a1psum_pool = tc.alloc_tile_pool(name="w1psum", bufs=1, space="PSUM")
        psum_w1 = a1psum_pool.tile([H, BL], f32, name="psum_w1")

        with (
            tc.tile_pool(name="scan_psum", bufs=1, space="PSUM") as scan_pp,
            tc.tile_pool(name="xs", bufs=3) as xs_pool,
            tc.tile_pool(name="acts", bufs=3) as act_pool,
            tc.tile_pool(name="dve", bufs=3) as dve_pool,
        ):
            # shared gate psum: [128, parity(2), bank(2), step(4), batch(128)]
            # each (parity, bank) slot is one full 2KB PSUM bank so the
            # start=True xproj matmul's bank-granular pending-zero covers
            # exactly what it writes.
            psum_g = scan_pp.tile([128, 2, 2, CH, BL], f32, name="psum_g")

            xs_tiles = {}
            s_last = [None]

            def emit_dma_chunk(dchunk):
                t0 = dchunk * DCH
                xs = xs_pool.tile(
                    [FA, DCH, BL], bf16, tag="xs", name=f"xs_{dchunk}"
                )
                nc.sync.dma_start(
                    out=xs[:, :, :].rearrange("f t b -> f (t b)"),
                    in_=xpat_d[:, t0 * BL : (t0 + DCH) * BL],
                )
                xs_tiles[dchunk] = xs

            def emit_xproj(chunk, bank):
                # whole-bank xproj (start=True) then K=1 bias broadcast;
                # one bank at a time so the PE burst spreads across steps
                par = chunk % 2
                xs = xs_tiles[chunk * CH // DCH]
                q0 = (chunk * CH) % DCH
                rhs = xs[:, q0 : q0 + CH, :]
                nc.tensor.matmul(
                    psum_g[:, par, bank, :, :],
                    lhsT=Wx_sb[:, bank * 128 : (bank + 1) * 128],
                    rhs=rhs,
                    start=True,
                    stop=False,
                    skip_group_check=True,
                )
                nc.tensor.matmul(
                    psum_g[:, par, bank, :, :],
                    lhsT=b_sb[0:1, bank * 128 : (bank + 1) * 128],
                    rhs=ones_sb[0:1, :].rearrange("p (t b) -> p t b", t=CH),
                    start=False,
                    stop=False,
                    skip_group_check=True,
                )

            emit_dma_chunk(0)
            emit_xproj(0, 0)
            emit_xproj(0, 1)
            pend_w1 = [None] * NG
            pend_h = [None] * NG
            s_t = [[None] * NG for _ in range(T)]
            m1_t = [[None] * NG for _ in range(T)]
            m2s_t = [[None] * NG for _ in range(T)]

            def emit_front(g, t):
                # pending h-mul from the previous step first: its tanh is
                # long done by now, so the DVE picks it up without blocking
                # the other chain's queued ops
                if pend_h[g] is not None:
                    sp, thp, hdst = pend_h[g]
                    nc.vector.tensor_mul(
                        hdst[64:128, :], sp[64:128, 1, :], thp[64:128, :]
                    )
                    pend_h[g] = None
                # delayed W1 accumulation + recurrent matmuls + sigmoid
                par = (t // CH) % 2
                ph = t % CH
                h_cur = h_ext[g][t % 2]
                for bank in range(2):
                    nc.tensor.matmul(
                        psum_g[:, par, bank, ph, g * BG : (g + 1) * BG],
                        lhsT=Whh[64:128, bank * 128 : (bank + 1) * 128],
                        rhs=h_cur[64:128, :],
                        start=False,
                        stop=(t % CH == CH - 1 and g == NG - 1),
                        skip_group_check=True,
                        tile_position=(64, 0),
                    )
                if pend_w1[g] is not None:
                    tw, hw = pend_w1[g]
                    nc.tensor.matmul(
                        psum_w1[0:H, g * BG : (g + 1) * BG],
                        lhsT=W1_sb[64:128, tw * H : (tw + 1) * H],
                        rhs=hw[64:128, :],
                        start=(tw == 0 and g == 0),
                        stop=False,
                        skip_group_check=True,
                        tile_position=(64, 0),
                    )
                s = act_pool.tile([128, 2, BG], f32, tag=f"s{g}", name=f"s{g}_{t}")
                s_last[0] = s
                nc.scalar.activation(
                    out=s[:],
                    in_=psum_g[:, par, :, ph, g * BG : (g + 1) * BG],
                    func=AF.Sigmoid,
                )
                s_t[t][g] = s

            def emit_mid(g, t):
                # m1 on gpsimd; m2 + shuffle on DVE (parallel paths)
                s = s_t[t][g]
                m1 = dve_pool.tile([128, BG], f32, tag=f"m1{g}", name=f"m1{g}_{t}")
                nc.gpsimd.tensor_mul(m1[64:128, :], s[64:128, 0, :], c_t[g])
                m2 = dve_pool.tile([H, BG], f32, tag=f"m2{g}", name=f"m2{g}_{t}")
                nc.vector._custom_dve(
                    AFFINE_MUL_REDUCE,
                    out=m2[:],
                    in0=s[0:H, 1, :],
                    in1=s[0:H, 0, :],
                    s0=2.0,
                    s1=-1.0,
                )
                m2s = dve_pool.tile([128, BG], f32, tag=f"m2s{g}", name=f"m2s{g}_{t}")
                nc.vector.stream_shuffle(m2s[64:128, :], m2[:], mask=list(range(32)))
                m1_t[t][g] = m1
                m2s_t[t][g] = m2s

            def emit_tail(g, t):
                s = s_t[t][g]
                nc.vector.tensor_add(
                    c_t[g], m1_t[t][g][64:128, :], m2s_t[t][g][64:128, :]
                )
                th = dve_pool.tile([128, BG], f32, tag=f"th{g}", name=f"th{g}_{t}")
                nc.scalar.activation(out=th[64:128, :], in_=c_t[g], func=AF.Tanh)
                h_nxt = h_ext[g][(t + 1) % 2]
                pend_h[g] = (s, th, h_nxt)
                if t == T - 1:
                    nc.vector.tensor_mul(hf[g], s[64:128, 1, :], th[64:128, :])
                pend_w1[g] = (t, h_nxt)
                s_t[t][g] = None
                m1_t[t][g] = None
                m2s_t[t][g] = None

            # skewed multi-chain pipeline: the second half of the groups
            # runs half a step behind the first half, so each in-order
            # engine stream alternates stages of different chains and no
            # chain queues behind another chain's later stages.
            for t in range(T):
                if t % DCH == 0 and (t // DCH) + 1 < T // DCH:
                    emit_dma_chunk(t // DCH + 1)
                if t % CH in (0, 2) and (t // CH) + 1 < NCH:
                    emit_xproj(t // CH + 1, (t % CH) // 2)
                for g in range(NG):
                    emit_front(g, t)
                    gt = (g + NG // 2) % NG
                    tt = t - 1 if gt >= NG // 2 else t
                    if tt >= 0:
                        emit_tail(gt, tt)
                    emit_mid(g, t)
            for gt in range(NG // 2, NG):
                emit_tail(gt, T - 1)
            for g in range(NG):
                if pend_h[g] is not None:
                    sp, thp, hdst = pend_h[g]
                    nc.vector.tensor_mul(
                        hdst[64:128, :], sp[64:128, 1, :], thp[64:128, :]
                    )
                    pend_h[g] = None
            # flush the delayed W1 accumulations
            for g in range(NG):
                tw, hw = pend_w1[g]
                nc.tensor.matmul(
                    psum_w1[0:H, g * BG : (g + 1) * BG],
                    lhsT=W1_sb[64:128, tw * H : (tw + 1) * H],
                    rhs=hw[64:128, :],
                    start=False,
                    stop=(g == NG - 1),
                    skip_group_check=True,
                    tile_position=(64, 0),
                )

        # ---------------- heads + output ----------------
        with tc.tile_pool(name="epi_psum", bufs=1, space="PSUM") as epi_pp:
            a1 = persist.tile([H, BL], bf16, name="a1")
            nc.scalar.activation(out=a1[:], in_=psum_w1[:], func=AF.Relu, bias=b1_sb[:])
            psum_h2 = epi_pp.tile([H, BL], f32, name="psum_h2")
            nc.tensor.matmul(psum_h2[:], lhsT=W2_sb[:], rhs=a1[:], start=True, stop=True)
            a2 = persist.tile([H, BL], bf16, name="a2")
            nc.scalar.activation(out=a2[:], in_=psum_h2[:], func=AF.Relu, bias=b2_sb[:])
            psum_h3 = epi_pp.tile([2 * A, BL], f32, name="psum_h3")
            nc.tensor.matmul(psum_h3[:], lhsT=W3_sb[:], rhs=a2[:], start=True, stop=True)
            actorT = persist.tile([2 * A, BL], f32, name="actorT")
            nc.scalar.activation(
                out=actorT[:], in_=psum_h3[:], func=AF.Identity, bias=b3_sb[:]
            )

            out_sb = persist.tile([BL, OUTW], f32, name="out_sb")
            pt = epi_pp.tile([BL, 2 * A], f32, name="pt")
            nc.tensor.transpose(pt[:], actorT[:], ident[0 : 2 * A, 0 : 2 * A])
            nc.vector.tensor_copy(out_sb[:, 0 : 2 * A], pt[:])
            pc = epi_pp.tile([BL, H], f32, name="pc")
            pht = epi_pp.tile([BL, H], f32, name="pht")
            nc.tensor.transpose(
                pc[:], c_all[64:128, :], ident[64:128, :], tile_position=(64, 0)
            )
            nc.tensor.transpose(
                pht[:], hf_all[64:128, :], ident[64:128, :], tile_position=(64, 0)
            )
            nc.vector.tensor_copy(out_sb[:, 2 * A : 2 * A + H], pc[:])
            nc.vector.tensor_copy(out_sb[:, 2 * A + H : OUTW], pht[:])
            nc.sync.dma_start(out=out_d[:], in_=out_sb[:])
        a1psum_pool.release()
        persist.release()

    return nc


_CACHE = {}


def _get_nc():
    if "nc" not in _CACHE:
        nc = build()
        nc.finalize()
        _CACHE["nc"] = nc
    return _CACHE["nc"]


def _prep_core(inputs, i, shared):
    sl = slice(i * BL, (i + 1) * BL)
    x = np.asarray(inputs["x"][sl], dtype=np.float32)          # [BL, T, F]
    pa = np.asarray(inputs["previous_actions"][sl], np.float32)
    xpa = np.concatenate([x, pa], axis=2)                      # [BL, T, FA]
    xpat = np.ascontiguousarray(xpa.transpose(2, 1, 0)).astype(BF)  # [FA, T, BL]
    m = {"xpat": xpat.reshape(FA, T * BL)}
    m.update(shared)
    return m


def _prep_shared(inputs):
    Wx = np.asarray(inputs["Wx"], np.float32).copy()   # [128, 256]
    Wh = np.asarray(inputs["Wh"], np.float32).copy()   # [64, 256]
    bb = np.asarray(inputs["b"], np.float32).copy()    # [256]
    # sigmoid trick: scale g-gate columns by 2
    Wx[:, 2 * H : 3 * H] *= 2.0
    Wh[:, 2 * H : 3 * H] *= 2.0
    bb[2 * H : 3 * H] *= 2.0
    W1 = np.asarray(inputs["W1"], np.float32)          # [T*H, H]
    W1p = np.ascontiguousarray(
        W1.reshape(T, H, H).transpose(1, 0, 2).reshape(H, T * H)
    )
    return {
        "Wxp": Wx.astype(BF),
        "Whp": Wh.astype(BF),
        "bp": bb[None, :].astype(BF),
        "W1p": W1p.astype(BF),
        "W2p": np.asarray(inputs["W2"], np.float32).astype(BF),
        "W3p": np.asarray(inputs["W3"], np.float32).astype(BF),
        "b1": np.asarray(inputs["b1"], np.float32),
        "b2": np.asarray(inputs["b2"], np.float32),
        "b3": np.asarray(inputs["b3"], np.float32),
    }


def _run(inputs, trace=False):
    nc = _get_nc()
    shared = _prep_shared(inputs)
    in_maps = [_prep_core(inputs, i, shared) for i in range(NCORES)]
    res = run_bass_kernel_spmd(nc, in_maps, core_ids=list(range(NCORES)), trace=trace)
    outs = [np.asarray(res.results[i]["out"]) for i in range(NCORES)]
    full = np.concatenate(outs, axis=0)  # [B, OUTW]
    actor = np.ascontiguousarray(full[:, 0 : 2 * A])
    c = np.ascontiguousarray(full[:, 2 * A : 2 * A + H])
    h = np.ascontiguousarray(full[:, 2 * A + H : OUTW])
    return (actor, (c, h)), res


def kernel(**inputs):
    (actor, (c, h)), _ = _run(inputs, trace=False)
    return actor, (c, h)
